# revision 11
# baseline (speedup 1.0000x reference)
"""GPT-OSS expert MLP (gate/up GEMM + clamped GLU + down GEMM + routing scale)
on 8 Trainium2 NeuronCores.

Sharding: tensor-parallel split of the intermediate dim I=2880 across 8 cores
(360 columns each, padded to 384 = 3*128). Each core computes
  gate/up = hidden @ W[:, slice] ; glu ; y_partial = glu_h @ down_w[slice, :]
and writes its full [H, T] partial (transposed layout). The host sums the 8
partials, applies down bias, routing weights, and the residual add.

All matmul operands are bf16: the quantized weights (values k/32, |k|<=4) are
exactly representable in bf16, so the only rounding is on hidden_states.
PSUM accumulation is fp32 and partials are written out in fp32.
"""

import numpy as np
import ml_dtypes

BF16 = ml_dtypes.bfloat16

H = 2880          # hidden size
I = 2880          # intermediate size
T = 512           # tokens
NCORES = 8
IC = I // NCORES  # 360 intermediate cols per core
ICP = 384         # padded to 3 * 128
MT = ICP // 128   # 3 i-tiles per core
HP = 2944         # H padded to 23 * 128
KT = HP // 128    # 23 k-tiles over hidden dim
ALPHA = 1.702
LIMIT = 7.0
# hidden k-chunk sizes (in k-tiles) for pipelined loading
HID_CHUNKS = [6, 6, 6, 5]

_cache = {}


def build_program(loop_reps=None):
    """Build (and compile) the per-core Bass program. Identical on all cores;
    per-core data comes from in_maps. If loop_reps is given, the body is
    wrapped in a hardware For_i loop (used only for timing)."""
    import concourse.bacc as bacc
    import concourse.mybir as mybir
    import concourse.tile as tile

    fp32 = mybir.dt.float32
    bf16 = mybir.dt.bfloat16

    nc = bacc.Bacc("TRN2", target_bir_lowering=False, debug=False,
                   num_devices=NCORES)

    hid_d = nc.dram_tensor("hid", [128, KT * T], bf16, kind="ExternalInput").ap()
    gu_d = nc.dram_tensor("gu", [128, 2 * MT * KT * 128], bf16,
                          kind="ExternalInput").ap()
    dw_d = nc.dram_tensor("dw", [128, KT * MT * 128], bf16,
                          kind="ExternalInput").ap()
    gb_d = nc.dram_tensor("gb", [128, MT], fp32, kind="ExternalInput").ap()
    ub_d = nc.dram_tensor("ub", [128, MT], fp32, kind="ExternalInput").ap()
    y_d = nc.dram_tensor("y", [HP, T], bf16, kind="ExternalOutput").ap()

    def body(ctx, tc):
        wpool = ctx.enter_context(tc.tile_pool(name="w", bufs=1))
        glupool = ctx.enter_context(tc.tile_pool(name="glu", bufs=2))
        psum = ctx.enter_context(
            tc.tile_pool(name="psum", bufs=2, space="PSUM"))
        psum_y = ctx.enter_context(
            tc.tile_pool(name="psum_y", bufs=4, space="PSUM"))

        # ---- loads, interleaved so PE's first needs arrive first ----
        # SP HWDGE ring is FIFO; emit in PE consumption order. Early pieces
        # are small so the PE can start ~3us in and m0-gate paces with the
        # hidden-state stream; later pieces are bigger (less issue overhead).
        hid_t = [None] * KT                    # kt -> (tile, col offset)
        gu_piece_sizes = {0: [6, 6, 6, 5], 1: [6, 6, 6, 5],
                          2: [12, 11], 3: [12, 11],
                          4: [12, 11], 5: [12, 11]}
        hid_piece_sizes = [2, 2, 2, 2, 3, 3, 3, 3, 3]
        gu_kt = {g: 0 for g in range(6)}       # next kt per group
        hid_kt = [0]
        gu_map = {}                            # (grp, kt) -> (tile, j)

        def load_hid():
            ci = sum(1 for k in range(KT) if hid_t[k] is not None)
            nk = hid_piece_sizes.pop(0)
            kt0 = hid_kt[0]
            t = wpool.tile([128, nk * T], bf16, tag=f"hid{ci}")
            nc.sync.dma_start(t[:], hid_d[:, kt0 * T:(kt0 + nk) * T])
            for j in range(nk):
                hid_t[kt0 + j] = (t, j)
            hid_kt[0] = kt0 + nk

        def load_gu_piece(grp, idx):
            nk = gu_piece_sizes[grp][idx]
            kt0 = gu_kt[grp]
            t = wpool.tile([128, nk * 128], bf16, tag=f"gu{grp}_{idx}")
            nc.sync.dma_start(
                t[:], gu_d[:, grp * KT * 128 + kt0 * 128:
                           grp * KT * 128 + (kt0 + nk) * 128])
            for j in range(nk):
                gu_map[(grp, kt0 + j)] = (t, j)
            gu_kt[grp] = kt0 + nk

        def gu_lhsT(grp, kt):
            t, j = gu_map[(grp, kt)]
            return t[:, j * 128:(j + 1) * 128]

        load_gu_piece(0, 0); load_hid(); load_gu_piece(0, 1); load_hid()
        load_gu_piece(0, 2); load_hid(); load_gu_piece(0, 3); load_hid()
        gb_t = wpool.tile([128, MT], fp32, tag="gb")
        nc.sync.dma_start(gb_t[:], gb_d[:])
        ub_t = wpool.tile([128, MT], fp32, tag="ub")
        nc.sync.dma_start(ub_t[:], ub_d[:])
        load_gu_piece(1, 0); load_hid(); load_gu_piece(1, 1); load_hid()
        load_gu_piece(1, 2); load_hid(); load_gu_piece(1, 3); load_hid()
        load_hid()
        for grp in (2, 3, 4, 5):
            load_gu_piece(grp, 0); load_gu_piece(grp, 1)

        dw_t = wpool.tile([128, KT * MT * 128], bf16, tag="dw")
        nc.sync.dma_start(dw_t[:], dw_d[:])

        hglu = wpool.tile([128, MT * T], bf16, tag="hglu")

        def rhs(kt):
            t, j = hid_t[kt]
            return t[:, j * T:(j + 1) * T]

        # ---- gate/up GEMMs + GLU per i-tile ----
        for m in range(MT):
            pg = psum.tile([128, T], fp32, tag="pg")
            for kt in range(KT):
                nc.tensor.matmul(pg[:], gu_lhsT(2 * m, kt),
                                 rhs(kt), start=(kt == 0), stop=(kt == KT - 1))
            pu = psum.tile([128, T], fp32, tag="pu")
            for kt in range(KT):
                nc.tensor.matmul(pu[:], gu_lhsT(2 * m + 1, kt),
                                 rhs(kt), start=(kt == 0), stop=(kt == KT - 1))

            # gate path: g = min(pg + gb, LIMIT); s = silu(ALPHA*g) = ALPHA*glu
            tg = glupool.tile([128, T], fp32, tag="tg")
            nc.vector.tensor_scalar(tg[:], pg[:], gb_t[:, m:m + 1], LIMIT,
                                    mybir.AluOpType.add, mybir.AluOpType.min)
            sg = glupool.tile([128, T], fp32, tag="sg")
            nc.scalar.activation(sg[:], tg[:],
                                 mybir.ActivationFunctionType.Silu, scale=ALPHA)
            # up path: u = clip(pu + ub, -LIMIT, LIMIT); u3 = (u + 1)/ALPHA
            tu = glupool.tile([128, T], fp32, tag="tu")
            nc.vector.tensor_scalar(tu[:], pu[:], ub_t[:, m:m + 1], LIMIT,
                                    mybir.AluOpType.add, mybir.AluOpType.min)
            tu3 = glupool.tile([128, T], fp32, tag="tu3")
            nc.vector.tensor_scalar(tu3[:], tu[:], -LIMIT, 1.0 / ALPHA,
                                    mybir.AluOpType.max, mybir.AluOpType.mult)
            tu4 = glupool.tile([128, T], fp32, tag="tu4")
            nc.vector.tensor_scalar_add(tu4[:], tu3[:], 1.0 / ALPHA)
            # h = (ALPHA*glu) * (u+1)/ALPHA = glu * (u + 1)
            nc.vector.tensor_tensor(hglu[:, m * T:(m + 1) * T], sg[:], tu4[:],
                                    mybir.AluOpType.mult)

        # ---- down GEMM, write bf16 partial y^T ----
        # stores batched 4 h-tiles per DMA: per-store issue cost (~650ns SP
        # sequencer + ~625ns HWDGE) would otherwise pace the whole tail
        ypool = ctx.enter_context(tc.tile_pool(name="yout", bufs=3))
        batches = [4, 4, 4, 4, 4, 2, 1]
        batch_start = 0
        yo = None
        for ht in range(KT):
            py = psum_y.tile([128, T], fp32, tag="py")
            for it in range(MT):
                nc.tensor.matmul(
                    py[:],
                    dw_t[:, ht * ICP + it * 128: ht * ICP + (it + 1) * 128],
                    hglu[:, it * T:(it + 1) * T],
                    start=(it == 0), stop=(it == MT - 1))
            bi = ht - batch_start
            if bi == 0:
                nb = batches[0]
                yo = ypool.tile([128, nb * T], bf16, tag="yo")
            # alternate PSUM->SBUF copies between DVE and ACT so the copy
            # stream keeps pace with the PE (one copy per ~650ns h-tile)
            if ht % 2 == 0:
                nc.vector.tensor_copy(yo[:, bi * T:(bi + 1) * T], py[:])
            else:
                nc.scalar.copy(yo[:, bi * T:(bi + 1) * T], py[:])
            if bi == nb - 1:
                h0 = ht - bi
                dst = y_d[h0 * 128:(h0 + nb) * 128, :].rearrange(
                    "(a p) t -> p a t", p=128)
                src_ap = yo[:].rearrange("p (a t) -> p a t", a=nb)
                nc.sync.dma_start(dst, src_ap)
                batches.pop(0)
                batch_start = ht + 1

    from contextlib import ExitStack
    with tile.TileContext(nc) as tc:
        with ExitStack() as ctx:
            if loop_reps is None:
                body(ctx, tc)
            else:
                with tc.For_i(0, loop_reps, 1,
                              hint_engines=(mybir.EngineType.PE,)):
                    body(ctx, tc)

    nc.compile()
    return nc


def prepare_in_maps(hidden_states, gate_w, gate_b, up_w, up_b, down_w):
    """Host-side shard + pad + pre-tile into the exact SBUF layouts."""
    hs = np.asarray(hidden_states, np.float32)
    hidT = np.zeros((HP, T), np.float32)
    hidT[:H] = hs.T
    hid_tiled = np.ascontiguousarray(
        hidT.astype(BF16).reshape(KT, 128, T).transpose(1, 0, 2)
    ).reshape(128, KT * T)

    gw = np.asarray(gate_w, np.float32)
    uw = np.asarray(up_w, np.float32)
    dwf = np.asarray(down_w, np.float32)
    gbf = np.asarray(gate_b, np.float32).reshape(-1)
    ubf = np.asarray(up_b, np.float32).reshape(-1)

    def lhsT_tiles(Wp):  # [HP, 128] -> [128, KT*128]
        return np.ascontiguousarray(
            Wp.reshape(KT, 128, 128).transpose(1, 0, 2)).reshape(128, KT * 128)

    in_maps = []
    for c in range(NCORES):
        sl = slice(c * IC, (c + 1) * IC)
        Gp = np.zeros((HP, ICP), np.float32)
        Gp[:H, :IC] = gw[:, sl]
        Up = np.zeros((HP, ICP), np.float32)
        Up[:H, :IC] = uw[:, sl]
        Gp = Gp.astype(BF16)
        Up = Up.astype(BF16)
        blocks = []
        for m in range(MT):
            blocks.append(lhsT_tiles(Gp[:, m * 128:(m + 1) * 128]))
            blocks.append(lhsT_tiles(Up[:, m * 128:(m + 1) * 128]))
        gu = np.ascontiguousarray(np.concatenate(blocks, axis=1))

        Dp = np.zeros((ICP, HP), np.float32)
        Dp[:IC, :H] = dwf[sl, :]
        dw_tiled = np.ascontiguousarray(
            Dp.astype(BF16).reshape(MT, 128, KT, 128).transpose(1, 2, 0, 3)
        ).reshape(128, KT * MT * 128)

        gbp = np.zeros(ICP, np.float32)
        gbp[:IC] = gbf[sl]
        ubp = np.zeros(ICP, np.float32)
        ubp[:IC] = ubf[sl]

        in_maps.append({
            "hid": hid_tiled,
            "gu": gu,
            "dw": dw_tiled,
            "gb": np.ascontiguousarray(gbp.reshape(MT, 128).T),
            "ub": np.ascontiguousarray(ubp.reshape(MT, 128).T),
        })
    return in_maps


def kernel(hidden_states, routing_weights, final_hidden_states,
           gate_w, gate_b, up_w, up_b, down_w, down_b, expert_mask):
    from concourse.bass_utils import run_bass_kernel_spmd

    if "nc" not in _cache:
        _cache["nc"] = build_program()
    nc = _cache["nc"]

    in_maps = prepare_in_maps(hidden_states, gate_w, gate_b, up_w, up_b, down_w)
    res = run_bass_kernel_spmd(nc, in_maps, list(range(NCORES)))

    ysum = np.zeros((HP, T), np.float64)
    for c in range(NCORES):
        ysum += res.results[c]["y"].astype(np.float64)
    y = ysum[:H].T.astype(np.float32)  # [T, H]

    mask = np.asarray(expert_mask, np.float32)          # [TOPK, T]
    rw = np.asarray(routing_weights, np.float32)        # [T, TOPK]
    tok_w = np.einsum("jt,tj->t", mask, rw)             # [T]

    out = (np.asarray(final_hidden_states, np.float32)
           + (y + np.asarray(down_b, np.float32).reshape(1, -1))
           * tok_w[:, None])
    return out.astype(np.float32)


# revision 14
# speedup vs baseline: 1.0942x; 1.0942x over previous
"""GPT-OSS expert MLP (gate/up GEMM + clamped GLU + down GEMM + routing scale)
on 8 Trainium2 NeuronCores.

Sharding: tensor-parallel split of the intermediate dim I=2880 across 8 cores
(360 columns each, padded to 384 = 3*128). Each core computes
  gate/up = hidden @ W[:, slice] ; glu ; y_partial = glu_h @ down_w[slice, :]
and writes its full [H, T] partial (transposed layout). The host sums the 8
partials, applies down bias, routing weights, and the residual add.

All matmul operands are bf16: the quantized weights (values k/32, |k|<=4) are
exactly representable in bf16, so the only rounding is on hidden_states.
PSUM accumulation is fp32; partials are written out in bf16 and
summed on the host in fp64.
"""

import numpy as np
import ml_dtypes

BF16 = ml_dtypes.bfloat16

H = 2880          # hidden size
I = 2880          # intermediate size
T = 512           # tokens
NCORES = 8
IC = I // NCORES  # 360 intermediate cols per core
ICP = 384         # padded to 3 * 128
MT = ICP // 128   # 3 i-tiles per core
HP = 2944         # H padded to 23 * 128
KT = HP // 128    # 23 k-tiles over hidden dim
ALPHA = 1.702
LIMIT = 7.0
_cache = {}


def build_program(loop_reps=None):
    """Build (and compile) the per-core Bass program. Identical on all cores;
    per-core data comes from in_maps. If loop_reps is given, the body is
    wrapped in a hardware For_i loop (used only for timing)."""
    import concourse.bacc as bacc
    import concourse.mybir as mybir
    import concourse.tile as tile

    fp32 = mybir.dt.float32
    bf16 = mybir.dt.bfloat16

    nc = bacc.Bacc("TRN2", target_bir_lowering=False, debug=False,
                   num_devices=NCORES)

    hid_d = nc.dram_tensor("hid", [128, KT * T], bf16, kind="ExternalInput").ap()
    gu_d = nc.dram_tensor("gu", [128, 2 * MT * KT * 128], bf16,
                          kind="ExternalInput").ap()
    dw_d = nc.dram_tensor("dw", [128, KT * MT * 128], bf16,
                          kind="ExternalInput").ap()
    gb_d = nc.dram_tensor("gb", [128, MT], fp32, kind="ExternalInput").ap()
    ub_d = nc.dram_tensor("ub", [128, MT], fp32, kind="ExternalInput").ap()
    y_d = nc.dram_tensor("y", [HP, T], bf16, kind="ExternalOutput").ap()

    def body(ctx, tc):
        wpool = ctx.enter_context(tc.tile_pool(name="w", bufs=1))
        glupool = ctx.enter_context(tc.tile_pool(name="glu", bufs=3))
        psum = ctx.enter_context(
            tc.tile_pool(name="psum", bufs=2, space="PSUM"))
        psum_y = ctx.enter_context(
            tc.tile_pool(name="psum_y", bufs=4, space="PSUM"))

        # ---- loads, interleaved so PE's first needs arrive first ----
        # SP HWDGE ring is FIFO; emit in PE consumption order. Early pieces
        # are small so the PE can start ~3us in and m0-gate paces with the
        # hidden-state stream; later pieces are bigger (less issue overhead).
        hid_t = [None] * KT                    # kt -> (tile, col offset)
        gu_piece_sizes = {0: [6, 6, 6, 5], 1: [6, 6, 6, 5],
                          2: [12, 11], 3: [12, 11],
                          4: [12, 11], 5: [12, 11]}
        hid_piece_sizes = [2, 2, 2, 2, 3, 3, 3, 3, 3]
        gu_kt = {g: 0 for g in range(6)}       # next kt per group
        hid_kt = [0]
        gu_map = {}                            # (grp, kt) -> (tile, j)

        def load_hid():
            ci = sum(1 for k in range(KT) if hid_t[k] is not None)
            nk = hid_piece_sizes.pop(0)
            kt0 = hid_kt[0]
            t = wpool.tile([128, nk * T], bf16, tag=f"hid{ci}")
            nc.sync.dma_start(t[:], hid_d[:, kt0 * T:(kt0 + nk) * T])
            for j in range(nk):
                hid_t[kt0 + j] = (t, j)
            hid_kt[0] = kt0 + nk

        def load_gu_piece(grp, idx):
            nk = gu_piece_sizes[grp][idx]
            kt0 = gu_kt[grp]
            t = wpool.tile([128, nk * 128], bf16, tag=f"gu{grp}_{idx}")
            nc.sync.dma_start(
                t[:], gu_d[:, grp * KT * 128 + kt0 * 128:
                           grp * KT * 128 + (kt0 + nk) * 128])
            for j in range(nk):
                gu_map[(grp, kt0 + j)] = (t, j)
            gu_kt[grp] = kt0 + nk

        def gu_lhsT(grp, kt):
            t, j = gu_map[(grp, kt)]
            return t[:, j * 128:(j + 1) * 128]

        load_gu_piece(0, 0); load_hid(); load_gu_piece(0, 1); load_hid()
        load_gu_piece(0, 2); load_hid(); load_gu_piece(0, 3); load_hid()
        gb_t = wpool.tile([128, MT], fp32, tag="gb")
        nc.sync.dma_start(gb_t[:], gb_d[:])
        ub_t = wpool.tile([128, MT], fp32, tag="ub")
        nc.sync.dma_start(ub_t[:], ub_d[:])
        load_gu_piece(1, 0); load_hid(); load_gu_piece(1, 1); load_hid()
        load_gu_piece(1, 2); load_hid(); load_gu_piece(1, 3); load_hid()
        load_hid()
        for grp in (2, 3, 4, 5):
            load_gu_piece(grp, 0); load_gu_piece(grp, 1)

        dw_t = wpool.tile([128, KT * MT * 128], bf16, tag="dw")
        nc.sync.dma_start(dw_t[:], dw_d[:])

        hglu = wpool.tile([128, MT * T], bf16, tag="hglu")

        def rhs(kt):
            t, j = hid_t[kt]
            return t[:, j * T:(j + 1) * T]

        # ---- gate/up GEMMs + GLU per i-tile ----
        for m in range(MT):
            pg = psum.tile([128, T], fp32, tag="pg")
            for kt in range(KT):
                nc.tensor.matmul(pg[:], gu_lhsT(2 * m, kt),
                                 rhs(kt), start=(kt == 0), stop=(kt == KT - 1))
            pu = psum.tile([128, T], fp32, tag="pu")
            for kt in range(KT):
                nc.tensor.matmul(pu[:], gu_lhsT(2 * m + 1, kt),
                                 rhs(kt), start=(kt == 0), stop=(kt == KT - 1))

            # gate path: g = min(pg + gb, LIMIT); s = silu(ALPHA*g) = ALPHA*glu
            tg = glupool.tile([128, T], fp32, tag="tg")
            nc.vector.tensor_scalar(tg[:], pg[:], gb_t[:, m:m + 1], LIMIT,
                                    mybir.AluOpType.add, mybir.AluOpType.min)
            sg = glupool.tile([128, T], fp32, tag="sg")
            nc.scalar.activation(sg[:], tg[:],
                                 mybir.ActivationFunctionType.Silu, scale=ALPHA)
            # up path: u = clip(pu + ub, -LIMIT, LIMIT); u3 = (u + 1)/ALPHA
            tu = glupool.tile([128, T], fp32, tag="tu")
            nc.vector.tensor_scalar(tu[:], pu[:], ub_t[:, m:m + 1], LIMIT,
                                    mybir.AluOpType.add, mybir.AluOpType.min)
            tu3 = glupool.tile([128, T], fp32, tag="tu3")
            nc.vector.tensor_scalar(tu3[:], tu[:], -LIMIT, 1.0 / ALPHA,
                                    mybir.AluOpType.max, mybir.AluOpType.mult)
            tu4 = glupool.tile([128, T], fp32, tag="tu4")
            nc.vector.tensor_scalar_add(tu4[:], tu3[:], 1.0 / ALPHA)
            # h = (ALPHA*glu) * (u+1)/ALPHA = glu * (u + 1)
            nc.vector.tensor_tensor(hglu[:, m * T:(m + 1) * T], sg[:], tu4[:],
                                    mybir.AluOpType.mult)

        # ---- down GEMM, write bf16 partial y^T ----
        # stores batched 4 h-tiles per DMA: per-store issue cost (~650ns SP
        # sequencer + ~625ns HWDGE) would otherwise pace the whole tail
        ypool = ctx.enter_context(tc.tile_pool(name="yout", bufs=3))
        batches = [4, 4, 4, 4, 4, 2, 1]
        batch_start = 0
        yo = None
        for ht in range(KT):
            py = psum_y.tile([128, T], fp32, tag="py")
            for it in range(MT):
                nc.tensor.matmul(
                    py[:],
                    dw_t[:, ht * ICP + it * 128: ht * ICP + (it + 1) * 128],
                    hglu[:, it * T:(it + 1) * T],
                    start=(it == 0), stop=(it == MT - 1))
            bi = ht - batch_start
            if bi == 0:
                nb = batches[0]
                yo = ypool.tile([128, nb * T], bf16, tag="yo")
            # alternate PSUM->SBUF copies between DVE and ACT so the copy
            # stream keeps pace with the PE (one copy per ~650ns h-tile)
            if ht % 2 == 0:
                nc.vector.tensor_copy(yo[:, bi * T:(bi + 1) * T], py[:])
            else:
                nc.scalar.copy(yo[:, bi * T:(bi + 1) * T], py[:])
            if bi == nb - 1:
                h0 = ht - bi
                dst = y_d[h0 * 128:(h0 + nb) * 128, :].rearrange(
                    "(a p) t -> p a t", p=128)
                src_ap = yo[:].rearrange("p (a t) -> p a t", a=nb)
                nc.sync.dma_start(dst, src_ap)
                batches.pop(0)
                batch_start = ht + 1

    from contextlib import ExitStack
    with tile.TileContext(nc) as tc:
        with ExitStack() as ctx:
            if loop_reps is None:
                body(ctx, tc)
            else:
                with tc.For_i(0, loop_reps, 1,
                              hint_engines=(mybir.EngineType.PE,)):
                    body(ctx, tc)

    nc.compile()
    return nc


def prepare_in_maps(hidden_states, gate_w, gate_b, up_w, up_b, down_w):
    """Host-side shard + pad + pre-tile into the exact SBUF layouts."""
    hs = np.asarray(hidden_states, np.float32)
    hidT = np.zeros((HP, T), np.float32)
    hidT[:H] = hs.T
    hid_tiled = np.ascontiguousarray(
        hidT.astype(BF16).reshape(KT, 128, T).transpose(1, 0, 2)
    ).reshape(128, KT * T)

    gw = np.asarray(gate_w, np.float32)
    uw = np.asarray(up_w, np.float32)
    dwf = np.asarray(down_w, np.float32)
    gbf = np.asarray(gate_b, np.float32).reshape(-1)
    ubf = np.asarray(up_b, np.float32).reshape(-1)

    def lhsT_tiles(Wp):  # [HP, 128] -> [128, KT*128]
        return np.ascontiguousarray(
            Wp.reshape(KT, 128, 128).transpose(1, 0, 2)).reshape(128, KT * 128)

    in_maps = []
    for c in range(NCORES):
        sl = slice(c * IC, (c + 1) * IC)
        Gp = np.zeros((HP, ICP), np.float32)
        Gp[:H, :IC] = gw[:, sl]
        Up = np.zeros((HP, ICP), np.float32)
        Up[:H, :IC] = uw[:, sl]
        Gp = Gp.astype(BF16)
        Up = Up.astype(BF16)
        blocks = []
        for m in range(MT):
            blocks.append(lhsT_tiles(Gp[:, m * 128:(m + 1) * 128]))
            blocks.append(lhsT_tiles(Up[:, m * 128:(m + 1) * 128]))
        gu = np.ascontiguousarray(np.concatenate(blocks, axis=1))

        Dp = np.zeros((ICP, HP), np.float32)
        Dp[:IC, :H] = dwf[sl, :]
        dw_tiled = np.ascontiguousarray(
            Dp.astype(BF16).reshape(MT, 128, KT, 128).transpose(1, 2, 0, 3)
        ).reshape(128, KT * MT * 128)

        gbp = np.zeros(ICP, np.float32)
        gbp[:IC] = gbf[sl]
        ubp = np.zeros(ICP, np.float32)
        ubp[:IC] = ubf[sl]

        in_maps.append({
            "hid": hid_tiled,
            "gu": gu,
            "dw": dw_tiled,
            "gb": np.ascontiguousarray(gbp.reshape(MT, 128).T),
            "ub": np.ascontiguousarray(ubp.reshape(MT, 128).T),
        })
    return in_maps


def kernel(hidden_states, routing_weights, final_hidden_states,
           gate_w, gate_b, up_w, up_b, down_w, down_b, expert_mask):
    from concourse.bass_utils import run_bass_kernel_spmd

    if "nc" not in _cache:
        _cache["nc"] = build_program()
    nc = _cache["nc"]

    in_maps = prepare_in_maps(hidden_states, gate_w, gate_b, up_w, up_b, down_w)
    res = run_bass_kernel_spmd(nc, in_maps, list(range(NCORES)))

    ysum = np.zeros((HP, T), np.float64)
    for c in range(NCORES):
        ysum += res.results[c]["y"].astype(np.float64)
    y = ysum[:H].T.astype(np.float32)  # [T, H]

    mask = np.asarray(expert_mask, np.float32)          # [TOPK, T]
    rw = np.asarray(routing_weights, np.float32)        # [T, TOPK]
    tok_w = np.einsum("jt,tj->t", mask, rw)             # [T]

    out = (np.asarray(final_hidden_states, np.float32)
           + (y + np.asarray(down_b, np.float32).reshape(1, -1))
           * tok_w[:, None])
    return out.astype(np.float32)
